# revision 13
# baseline (speedup 1.0000x reference)
"""Trainium2 Bass kernel for nn_BackboneRITS (RITS recurrent imputation cell).

Strategy:
  - Data-parallel over batch: B=1024 -> 128 per core x 8 cores; weights replicated.
  - Everything on-device is *feature-major* ([feature, batch] per timestep), so
    LSTM/RITS matmuls need no transposes anywhere: out_chunk[Mfeat,B] =
    W_slab[K,M].T @ act[K,B], with fp16 operands (1 cyc/row on PE) and fp32 PSUM
    accumulation.  Biases are injected with K=1 rank-1 matmuls (bias x ones).
  - gamma_h / gamma_x / alpha depend only on inputs (d, m) -> computed off the
    critical recurrence chain, pipelined one step ahead.
  - Gate rows are pre-permuted [i,f,o,g] so one batched Sigmoid covers i,f,o.
  - imputed == c_c == m*x + (1-m)*c_h, so it is reassembled on the host from CH
    (exact elementwise, no extra device work).
"""

import os
import sys

sys.path.insert(0, "/opt/trn_rl_repo")
os.environ.setdefault("MYCRO_LOCAL_CACHE", "1")

import numpy as np

import concourse.bacc as bacc
import concourse.tile as tile
from concourse import mybir
from concourse.bass_utils import run_bass_kernel_spmd

B = 128          # batch per core
T = 128          # timesteps
F = 128          # features
H = 512          # hidden
G = 2048         # 4*H gate width
HC = H // 128    # hidden chunks
GC = G // 128    # gate chunks

F16 = mybir.dt.float16
F32 = mybir.dt.float32
AF = mybir.ActivationFunctionType
OP = mybir.AluOpType


def build_nc(n_steps=T):
    nc = bacc.Bacc("TRN2", target_bir_lowering=False, debug=False,
                   enable_asserts=True, num_devices=8)

    # ---- DRAM tensors -------------------------------------------------------
    def din(name, shape, dt=F16):
        return nc.dram_tensor(name, shape, dt, kind="ExternalInput").ap()

    def dout(name, shape, dt=F32):
        return nc.dram_tensor(name, shape, dt, kind="ExternalOutput").ap()

    xT_d = din("xT", [n_steps, F, B])
    mT_d = din("mT", [n_steps, F, B])
    dT_d = din("dT", [n_steps, F, B])
    mT8_d = din("mT8", [n_steps, F, B], mybir.dt.uint8)
    whh_d = din("whh", [128, HC * G])      # W_hh.T row-slabs side by side
    wih_d = din("wih", [128, 2 * G])       # W_ih.T slabs (c_c | m)
    whist_d = din("whist", [128, HC * F])  # W_hist.T row-slabs
    wgh_d = din("wgh", [128, H])           # W_gh.T
    wfeat_d = din("wfeat", [128, F])       # (W_feat*(1-I)).T
    wcomb_d = din("wcomb", [128, 2 * F])   # W_comb.T slabs (gamma_x | m)
    biasg_d = din("biasg", [128, G])       # row0 = (b_ih+b_hh)[perm], rest 0
    biasgh_r_d = din("biasgh_r", [128, H]) # row0 = b_gh, rest 0
    e0_d = din("e0", [128, 128])           # row0 = ones, rest 0
    eye_d = din("eye16", [128, 128])       # fp16 identity for PE transpose
    biass_d = din("biass", [1, 896])       # b_hist | b_feat | b_gh(512) | b_comb
    wgx_d = din("wgx", [F, 1], F32)        # diag(W_gx) per-partition column
    nbgx_d = din("nbgx", [F, 1], F32)      # -b_gx per-partition column

    XH_d = dout("XH", [n_steps, F, B])
    ZH_d = dout("ZH", [n_steps, F, B])
    CH_d = dout("CH", [n_steps, F, B])
    HT_d = dout("HT", [128, H])            # final h, chunked feature-major

    from contextlib import ExitStack
    with tile.TileContext(nc) as tc, ExitStack() as ctx:
        wp = ctx.enter_context(tc.tile_pool(name="weights", bufs=1))
        inp = ctx.enter_context(tc.tile_pool(name="inputs", bufs=4))
        st = ctx.enter_context(tc.tile_pool(name="state", bufs=1))
        wk = ctx.enter_context(tc.tile_pool(name="work", bufs=2))
        ps_g = ctx.enter_context(tc.tile_pool(name="ps_gates", bufs=1, space="PSUM"))
        ps_gam = ctx.enter_context(tc.tile_pool(name="ps_gamtr", bufs=1, space="PSUM"))
        ps_xz = ctx.enter_context(tc.tile_pool(name="ps_xz", bufs=1, space="PSUM"))
        ps_tr2 = ctx.enter_context(tc.tile_pool(name="ps_tr2", bufs=1, space="PSUM"))
        ps_al = ctx.enter_context(tc.tile_pool(name="ps_alpha", bufs=1, space="PSUM"))

        # ---- resident weights ----------------------------------------------
        whh = wp.tile([128, HC * G], F16)
        nc.sync.dma_start(whh[:], whh_d[:])
        wih = wp.tile([128, 2 * G], F16)
        nc.sync.dma_start(wih[:], wih_d[:])
        whist = wp.tile([128, HC * F], F16)
        nc.sync.dma_start(whist[:], whist_d[:])
        wgh = wp.tile([128, H], F16)
        nc.sync.dma_start(wgh[:], wgh_d[:])
        wfeat = wp.tile([128, F], F16)
        nc.sync.dma_start(wfeat[:], wfeat_d[:])
        wcomb = wp.tile([128, 2 * F], F16)
        nc.sync.dma_start(wcomb[:], wcomb_d[:])
        biasg = wp.tile([128, G], F16)
        nc.sync.dma_start(biasg[:], biasg_d[:])
        biasgh_r = wp.tile([128, H], F16)
        nc.sync.dma_start(biasgh_r[:], biasgh_r_d[:])
        e0 = wp.tile([128, 128], F16)
        nc.sync.dma_start(e0[:], e0_d[:])
        eye16 = wp.tile([128, 128], F16)
        nc.sync.dma_start(eye16[:], eye_d[:])
        biass = wp.tile([1, 896], F16)
        nc.sync.dma_start(biass[:], biass_d[:])
        wgx = wp.tile([F, 1], F32)
        nc.sync.dma_start(wgx[:], wgx_d[:])
        nbgx = wp.tile([F, 1], F32)
        nc.sync.dma_start(nbgx[:], nbgx_d[:])
        ones = wp.tile([1, B], F16)
        nc.vector.memset(ones[:], 1.0)

        b_hist = biass[0:1, 0:128]
        b_feat = biass[0:1, 128:256]
        b_gh = biass[0:1, 256:768]
        b_comb = biass[0:1, 768:896]

        # ---- persistent state ----------------------------------------------
        hT = st.tile([128, H], F16)   # gamma-premultiplied h, chunked
        cT = st.tile([128, H], F16)
        nc.vector.memset(hT[:], 0.0)
        nc.vector.memset(cT[:], 0.0)

        # per-step rotating tiles, by tag
        x_in = [None] * n_steps
        m_in = [None] * n_steps
        m8_in = [None] * n_steps
        d_in = [None] * n_steps
        gam = [None] * n_steps     # gamma_h^T fp16 [128, 512]
        gamx = [None] * n_steps    # gamma_x^T fp16 [128, 128]
        alph = [None] * n_steps    # alpha^T fp16
        g_ps_t = [None] * n_steps  # gates psum (batch-major)
        xz_ps_t = [None] * n_steps
        al_ps_t = [None] * n_steps
        gam_ps_t = [None] * n_steps

        def emit_far(s):
            """input DMAs for step s + gamma_h psum matmuls for step s."""
            x_in[s] = inp.tile([F, B], F16, tag="x_in", name="x_in")
            nc.sync.dma_start(x_in[s][:], xT_d[s])
            m_in[s] = inp.tile([F, B], F16, tag="m_in", name="m_in")
            nc.sync.dma_start(m_in[s][:], mT_d[s])
            d_in[s] = inp.tile([F, B], F16, tag="d_in", name="d_in")
            nc.sync.dma_start(d_in[s][:], dT_d[s])
            m8_in[s] = inp.tile([F, B], mybir.dt.uint8, tag="m8_in", name="m8_in")
            nc.sync.dma_start(m8_in[s][:], mT8_d[s])
            if s >= 1:
                # gamma_h pre-activation, batch-major [B, 512]
                gp = ps_gam.tile([128, H], F32, tag="gamtr", name="gam_ps")
                gam_ps_t[s] = gp
                nc.tensor.matmul(gp[:], e0[:], biasgh_r[:], start=True, stop=False)
                nc.tensor.matmul(gp[:], d_in[s][:], wgh[:], start=False, stop=True)

        def emit_near_a(s):
            """input-only work for step s with no psum-bank contention:
            gamma_x chain, alpha, gamma_h activation."""
            # gamma_x chain (feature-major, per-partition scale/bias)
            y16 = wk.tile([F, B], F16, tag="y16", name="y16")
            nc.vector.tensor_scalar(y16[:], d_in[s][:], wgx[:, 0:1], None, OP.mult)
            e16 = wk.tile([F, B], F16, tag="e16", name="e16")
            nc.scalar.activation(e16[:], y16[:], AF.Exp, bias=nbgx[:, 0:1], scale=-1.0)
            gamx[s] = wk.tile([F, B], F16, tag="gamx", name="gamx")
            nc.vector.tensor_scalar(gamx[s][:], e16[:], 1.0, None, OP.min)

            # alpha matmuls are emitted mid-chain of the previous step (emit_mid)

            # gamma_h activation: min(exp(-(pre)), 1)
            if s >= 1:
                gexp = wk.tile([128, H], F16, tag="gexp", name="gexp")
                nc.scalar.activation(gexp[:], gam_ps_t[s][:], AF.Exp, scale=-1.0)
                gam[s] = wk.tile([128, H], F16, tag="gam", name="gam")
                nc.vector.tensor_scalar(gam[s][:], gexp[:], 1.0, None, OP.min)

        def emit_near_b(s):
            """psum-bank-reusing prep for step s (must be emitted after
            emit_chain(s-1) so the previous step's readers precede us)."""
            # gates psum (batch-major [B, 2048]): bias rows + m slabs
            gps = ps_g.tile([128, G], F32, tag="g_ps", name="g_ps")
            g_ps_t[s] = gps
            for c in range(4):
                nc.tensor.matmul(gps[:, c * 512:(c + 1) * 512],
                                 e0[:], biasg[:, c * 512:(c + 1) * 512],
                                 start=True, stop=False)
            for c in range(4):
                nc.tensor.matmul(gps[:, c * 512:(c + 1) * 512],
                                 m_in[s][:], wih[:, G + c * 512:G + (c + 1) * 512],
                                 start=False, stop=False)



        def emit_mid(s):
            """bias/alpha matmuls for step s, emitted mid-chain of step s-1 so
            they fill the PE stall while the previous step's DVE chain runs."""
            xz = ps_xz.tile([128, 256], F32, tag="xz_ps", name="xz_ps")
            xz_ps_t[s] = xz
            nc.tensor.matmul(xz[:, 0:128], b_hist[:], ones[:], start=True, stop=False)
            nc.tensor.matmul(xz[:, 128:256], b_feat[:], ones[:], start=False, stop=False,
                             skip_group_check=True)
            ap_ = ps_al.tile([128, B], F32, tag="al_ps", name="al_ps")
            al_ps_t[s] = ap_
            nc.tensor.matmul(ap_[:], b_comb[:], ones[:], start=True, stop=False)
            nc.tensor.matmul(ap_[:], wcomb[:, 0:F], gamx[s][:], start=False, stop=False)
            nc.tensor.matmul(ap_[:], wcomb[:, F:2 * F], m_in[s][:], start=False, stop=True)
            ta = wk.tile([F, B], F16, tag="ta", name="ta")
            nc.scalar.activation(ta[:], ap_[:], AF.Tanh, scale=0.5)
            alph[s] = wk.tile([F, B], F16, tag="alpha", name="alpha")
            nc.vector.tensor_scalar(alph[s][:], ta[:], 0.5, 0.5, OP.mult, OP.add)

        def emit_hwork(s, k):
            """x_h slab k + gate h-slabs for chunk k of step s (needs hT chunk k)."""
            nc.tensor.matmul(xz_ps_t[s][:, 0:128], whist[:, k * 128:(k + 1) * 128],
                             hT[:, k * 128:(k + 1) * 128], start=False, stop=(k == HC - 1))
            for c in range(4):
                nc.tensor.matmul(g_ps_t[s][:, c * 512:(c + 1) * 512],
                                 hT[:, k * 128:(k + 1) * 128],
                                 whh[:, k * G + c * 512:k * G + (c + 1) * 512],
                                 start=False, stop=False)

        def emit_chain(s):
            gps = g_ps_t[s]
            x_h_ps = xz_ps_t[s][:, 0:128]
            z_ps = xz_ps_t[s][:, 128:256]

            if s == 0:
                for k in range(HC):
                    emit_hwork(0, k)

            # x_h evac (fp16 for compute), x_c select
            x_h16 = wk.tile([F, B], F16, tag="x_h16", name="x_h16")
            nc.vector.tensor_copy(x_h16[:], x_h_ps)
            x_c16 = wk.tile([F, B], F16, tag="x_c16", name="x_c16")
            nc.vector.tensor_copy(x_c16[:], x_h16[:])
            nc.vector.copy_predicated(x_c16[:], m8_in[s][:], x_in[s][:])

            # z_h: x_c slab (skip group check: x_h group in this bank already closed)
            nc.tensor.matmul(z_ps, wfeat[:], x_c16[:], start=False, stop=True,
                             skip_group_check=True)

            # c_h = x_h + alpha*(z - x_h); CH output in fp32
            t1 = wk.tile([F, B], F16, tag="t1", name="t1")
            nc.vector.tensor_tensor(t1[:], z_ps, x_h16[:], OP.subtract)
            t2 = wk.tile([F, B], F16, tag="t2", name="t2")
            nc.vector.tensor_mul(t2[:], alph[s][:], t1[:])
            if s + 1 < n_steps:
                emit_mid(s + 1)
            c_h16 = wk.tile([F, B], F16, tag="c_h16", name="c_h16")
            nc.vector.tensor_add(c_h16[:], x_h16[:], t2[:])
            c_h32 = wk.tile([F, B], F32, tag="c_h32", name="c_h32")
            nc.vector.tensor_add(c_h32[:], x_h16[:], t2[:])

            # c_c select, gates c_c slab
            c_c16 = wk.tile([F, B], F16, tag="c_c16", name="c_c16")
            nc.vector.tensor_copy(c_c16[:], c_h16[:])
            nc.vector.copy_predicated(c_c16[:], m8_in[s][:], x_in[s][:])
            for c in range(4):
                nc.tensor.matmul(gps[:, c * 512:(c + 1) * 512],
                                 c_c16[:], wih[:, c * 512:(c + 1) * 512],
                                 start=False, stop=True)

            # activations: sigmoid over i|f|o, tanh over g
            tfo = wk.tile([128, 3 * H], F16, tag="tfo", name="tfo")
            nc.scalar.activation(tfo[:], gps[:, 0:3 * H], AF.Tanh, scale=0.5)
            sig = wk.tile([128, 3 * H], F16, tag="sig", name="sig")
            nc.vector.tensor_scalar(sig[:, H:2 * H], tfo[:, H:2 * H], 0.5, 0.5, OP.mult, OP.add)
            nc.vector.tensor_scalar(sig[:, 0:H], tfo[:, 0:H], 0.5, 0.5, OP.mult, OP.add)
            nc.vector.tensor_scalar(sig[:, 2 * H:3 * H], tfo[:, 2 * H:3 * H], 0.5, 0.5, OP.mult, OP.add)
            tg = wk.tile([128, H], F16, tag="tg", name="tg")
            nc.scalar.activation(tg[:], gps[:, 3 * H:G], AF.Tanh)
            if s + 1 < n_steps:
                emit_near_b(s + 1)

            # c_new = sig_f*c + sig_i*tanh_g
            fc = wk.tile([128, H], F16, tag="fc", name="fc")
            nc.vector.tensor_mul(fc[:], sig[:, H:2 * H], cT[:])
            ig = wk.tile([128, H], F16, tag="ig", name="ig")
            nc.vector.tensor_mul(ig[:], sig[:, 0:H], tg[:])
            nc.vector.tensor_add(cT[:], fc[:], ig[:])

            # h_new: batch-major s = sig_o * tanh(c), then PE-transpose back to
            # feature-major with the gamma(t+1) multiply fused into the evacuation
            thc = wk.tile([128, H], F16, tag="thc", name="thc")
            nc.scalar.activation(thc[:], cT[:], AF.Tanh)
            if s < n_steps - 1:
                s_pre = wk.tile([128, H], F16, tag="s_pre", name="s_pre")
                nc.vector.tensor_mul(s_pre[:], gam[s + 1][:], sig[:, 2 * H:3 * H])
                s_bm = wk.tile([128, H], F16, tag="s_bm", name="s_bm")
                trA = ps_gam.tile([128, 256], F16, tag="gamtr", name="trA")
                trB = ps_tr2.tile([128, 256], F16, tag="tr2", name="trB")
                trs = [(trA, 0), (trB, 0), (trA, 1), (trB, 1)]
                for k in range(HC):
                    nc.vector.tensor_mul(s_bm[:, k * 128:(k + 1) * 128],
                                         thc[:, k * 128:(k + 1) * 128],
                                         s_pre[:, k * 128:(k + 1) * 128])
                    tr, half = trs[k]
                    nc.tensor.matmul(tr[:, half * 128:(half + 1) * 128],
                                     s_bm[:, k * 128:(k + 1) * 128], eye16[:],
                                     is_transpose=True, start=(half == 0), stop=True,
                                     skip_group_check=(half == 1))
                    nc.vector.tensor_copy(hT[:, k * 128:(k + 1) * 128],
                                          tr[:, half * 128:(half + 1) * 128])
                    emit_hwork(s + 1, k)
            else:
                h32 = wk.tile([128, H], F32, tag="h32", name="h32")
                nc.vector.tensor_mul(h32[:], thc[:], sig[:, 2 * H:3 * H])
                nc.sync.dma_start(HT_d[:], h32[:])

            # fp32 output evacuations + DMA out
            x_h32 = wk.tile([F, B], F32, tag="x_h32", name="x_h32")
            nc.vector.tensor_copy(x_h32[:], x_h_ps)
            z32 = wk.tile([F, B], F32, tag="z32", name="z32")
            nc.vector.tensor_copy(z32[:], z_ps)
            nc.sync.dma_start(XH_d[s], x_h32[:])
            nc.sync.dma_start(ZH_d[s], z32[:])
            nc.sync.dma_start(CH_d[s], c_h32[:])

        # ---- program ---------------------------------------------------------
        emit_far(0)
        if n_steps > 1:
            emit_far(1)
        emit_near_a(0)
        emit_near_b(0)
        emit_mid(0)
        for t in range(n_steps):
            if t + 1 < n_steps:
                emit_near_a(t + 1)
            emit_chain(t)
            if t + 2 < n_steps:
                emit_far(t + 2)

    nc.compile()
    return nc


# ---- host-side prep ---------------------------------------------------------

def _prep_shared(W_gh, b_gh, W_gx, b_gx, W_hist, b_hist, W_feat, b_feat,
                 W_comb, b_comb, W_ih, W_hh, b_ih, b_hh):
    f16 = np.float16
    eye = np.eye(F, dtype=np.float32)
    perm = np.r_[0:512, 512:1024, 1536:2048, 1024:1536]  # [i,f,o,g]

    def slabify(WT, nslab):  # [nslab*128, N] -> [128, nslab*N]
        n = WT.shape[1]
        return np.ascontiguousarray(
            WT.reshape(nslab, 128, n).transpose(1, 0, 2).reshape(128, nslab * n))

    whh = slabify(np.ascontiguousarray(W_hh[perm].T), HC).astype(f16)
    wih = slabify(np.ascontiguousarray(W_ih[perm].T), 2).astype(f16)
    whist = slabify(np.ascontiguousarray(W_hist.T), HC).astype(f16)
    wgh = np.ascontiguousarray(W_gh.T).astype(f16)
    wfeat = np.ascontiguousarray((W_feat * (1.0 - eye)).T).astype(f16)
    wcomb = slabify(np.ascontiguousarray(W_comb.T), 2).astype(f16)
    biasg = np.zeros((128, G), np.float32)
    biasg[0] = (b_ih + b_hh)[perm]
    biasg = biasg.astype(f16)
    biasgh_r = np.zeros((128, H), np.float32)
    biasgh_r[0] = b_gh
    biasgh_r = biasgh_r.astype(f16)
    e0 = np.zeros((128, 128), np.float32)
    e0[0] = 1.0
    e0 = e0.astype(f16)
    eye16 = np.eye(128, dtype=np.float32).astype(f16)
    biass = np.concatenate([b_hist, b_feat, b_gh, b_comb]).reshape(1, 896).astype(f16)
    wgx = np.ascontiguousarray(np.diag(W_gx)).reshape(F, 1).astype(np.float32)
    nbgx = (-b_gx).reshape(F, 1).astype(np.float32)
    return dict(whh=whh, wih=wih, whist=whist, wgh=wgh, wfeat=wfeat,
                wcomb=wcomb, biasg=biasg, biasgh_r=biasgh_r, e0=e0,
                eye16=eye16, biass=biass, wgx=wgx, nbgx=nbgx)


_NC_CACHE = {}


def kernel(X, missing_mask, deltas, W_gh, b_gh, W_gx, b_gx, W_hist, b_hist,
           W_feat, b_feat, W_comb, b_comb, W_ih, W_hh, b_ih, b_hh):
    X = np.asarray(X, np.float32)
    missing_mask = np.asarray(missing_mask, np.float32)
    deltas = np.asarray(deltas, np.float32)

    n_steps = X.shape[1]
    if n_steps not in _NC_CACHE:
        _NC_CACHE[n_steps] = build_nc(n_steps)
    nc = _NC_CACHE[n_steps]

    shared = _prep_shared(np.asarray(W_gh, np.float32), np.asarray(b_gh, np.float32),
                          np.asarray(W_gx, np.float32), np.asarray(b_gx, np.float32),
                          np.asarray(W_hist, np.float32), np.asarray(b_hist, np.float32),
                          np.asarray(W_feat, np.float32), np.asarray(b_feat, np.float32),
                          np.asarray(W_comb, np.float32), np.asarray(b_comb, np.float32),
                          np.asarray(W_ih, np.float32), np.asarray(W_hh, np.float32),
                          np.asarray(b_ih, np.float32), np.asarray(b_hh, np.float32))

    n_cores = 8
    bt = X.shape[0] // n_cores
    in_maps = []
    for c in range(n_cores):
        sl = slice(c * bt, (c + 1) * bt)
        in_maps.append(dict(
            xT=np.ascontiguousarray(X[sl].transpose(1, 2, 0)).astype(np.float16),
            mT=np.ascontiguousarray(missing_mask[sl].transpose(1, 2, 0)).astype(np.float16),
            mT8=np.ascontiguousarray(missing_mask[sl].transpose(1, 2, 0)).astype(np.uint8),
            dT=np.ascontiguousarray(deltas[sl].transpose(1, 2, 0)).astype(np.float16),
            **shared,
        ))

    res = run_bass_kernel_spmd(nc, in_maps, core_ids=list(range(n_cores)))

    Bfull = X.shape[0]
    XH = np.empty((Bfull, n_steps, F), np.float32)
    ZH = np.empty_like(XH)
    CH = np.empty_like(XH)
    h_T = np.empty((Bfull, H), np.float32)
    for c in range(n_cores):
        r = res.results[c]
        sl = slice(c * bt, (c + 1) * bt)
        XH[sl] = r["XH"].transpose(2, 0, 1)
        ZH[sl] = r["ZH"].transpose(2, 0, 1)
        CH[sl] = r["CH"].transpose(2, 0, 1)
        h_T[sl] = r["HT"]

    imputed = missing_mask * X + (1.0 - missing_mask) * CH
    return imputed, CH, h_T, XH, CH, ZH


# revision 14
# speedup vs baseline: 1.0305x; 1.0305x over previous
"""Trainium2 Bass kernel for nn_BackboneRITS (RITS recurrent imputation cell).

Strategy:
  - Data-parallel over batch: B=1024 -> 128 per core x 8 cores; weights replicated.
  - Everything on-device is *feature-major* ([feature, batch] per timestep), so
    LSTM/RITS matmuls need no transposes anywhere: out_chunk[Mfeat,B] =
    W_slab[K,M].T @ act[K,B], with fp16 operands (1 cyc/row on PE) and fp32 PSUM
    accumulation.  Biases are injected with K=1 rank-1 matmuls (bias x ones).
  - gamma_h / gamma_x / alpha depend only on inputs (d, m) -> computed off the
    critical recurrence chain, pipelined one step ahead.
  - Gate rows are pre-permuted [i,f,o,g] so one batched Sigmoid covers i,f,o.
  - imputed == c_c == m*x + (1-m)*c_h, so it is reassembled on the host from CH
    (exact elementwise, no extra device work).
"""

import os
import sys

sys.path.insert(0, "/opt/trn_rl_repo")
os.environ.setdefault("MYCRO_LOCAL_CACHE", "1")

import numpy as np

import concourse.bacc as bacc
import concourse.tile as tile
from concourse import mybir
from concourse.bass_utils import run_bass_kernel_spmd

B = 128          # batch per core
T = 128          # timesteps
F = 128          # features
H = 512          # hidden
G = 2048         # 4*H gate width
HC = H // 128    # hidden chunks
GC = G // 128    # gate chunks

F16 = mybir.dt.float16
F32 = mybir.dt.float32
AF = mybir.ActivationFunctionType
OP = mybir.AluOpType


def build_nc(n_steps=T):
    nc = bacc.Bacc("TRN2", target_bir_lowering=False, debug=False,
                   enable_asserts=True, num_devices=8)

    # ---- DRAM tensors -------------------------------------------------------
    def din(name, shape, dt=F16):
        return nc.dram_tensor(name, shape, dt, kind="ExternalInput").ap()

    def dout(name, shape, dt=F32):
        return nc.dram_tensor(name, shape, dt, kind="ExternalOutput").ap()

    xT_d = din("xT", [n_steps, F, B])
    mT_d = din("mT", [n_steps, F, B])
    dT_d = din("dT", [n_steps, F, B])
    mT8_d = din("mT8", [n_steps, F, B], mybir.dt.uint8)
    whh_d = din("whh", [128, HC * G])      # W_hh.T row-slabs side by side
    wih_d = din("wih", [128, 2 * G])       # W_ih.T slabs (c_c | m)
    whist_d = din("whist", [128, HC * F])  # W_hist.T row-slabs
    wgh_d = din("wgh", [128, H])           # W_gh.T
    wfeat_d = din("wfeat", [128, F])       # (W_feat*(1-I)).T
    wcomb_d = din("wcomb", [128, 2 * F])   # W_comb.T slabs (gamma_x | m)
    biasg_d = din("biasg", [128, G])       # row0 = (b_ih+b_hh)[perm], rest 0
    biasgh_r_d = din("biasgh_r", [128, H]) # row0 = b_gh, rest 0
    e0_d = din("e0", [128, 128])           # row0 = ones, rest 0
    eye_d = din("eye16", [128, 128])       # fp16 identity for PE transpose
    biass_d = din("biass", [1, 896])       # b_hist | b_feat | b_gh(512) | b_comb
    wgx_d = din("wgx", [F, 1], F32)        # diag(W_gx) per-partition column
    nbgx_d = din("nbgx", [F, 1], F32)      # -b_gx per-partition column

    XH_d = dout("XH", [n_steps, F, B])
    ZH_d = dout("ZH", [n_steps, F, B])
    CH_d = dout("CH", [n_steps, F, B])
    HT_d = dout("HT", [128, H])            # final h, chunked feature-major

    from contextlib import ExitStack
    with tile.TileContext(nc) as tc, ExitStack() as ctx:
        wp = ctx.enter_context(tc.tile_pool(name="weights", bufs=1))
        inp = ctx.enter_context(tc.tile_pool(name="inputs", bufs=4))
        st = ctx.enter_context(tc.tile_pool(name="state", bufs=1))
        wk = ctx.enter_context(tc.tile_pool(name="work", bufs=2))
        ps_g = ctx.enter_context(tc.tile_pool(name="ps_gates", bufs=1, space="PSUM"))
        ps_gam = ctx.enter_context(tc.tile_pool(name="ps_gamtr", bufs=1, space="PSUM"))
        ps_xz = ctx.enter_context(tc.tile_pool(name="ps_xz", bufs=1, space="PSUM"))
        ps_tr2 = ctx.enter_context(tc.tile_pool(name="ps_tr2", bufs=1, space="PSUM"))
        ps_al = ctx.enter_context(tc.tile_pool(name="ps_alpha", bufs=1, space="PSUM"))

        # ---- resident weights ----------------------------------------------
        whh = wp.tile([128, HC * G], F16)
        nc.sync.dma_start(whh[:], whh_d[:])
        wih = wp.tile([128, 2 * G], F16)
        nc.sync.dma_start(wih[:], wih_d[:])
        whist = wp.tile([128, HC * F], F16)
        nc.sync.dma_start(whist[:], whist_d[:])
        wgh = wp.tile([128, H], F16)
        nc.sync.dma_start(wgh[:], wgh_d[:])
        wfeat = wp.tile([128, F], F16)
        nc.sync.dma_start(wfeat[:], wfeat_d[:])
        wcomb = wp.tile([128, 2 * F], F16)
        nc.sync.dma_start(wcomb[:], wcomb_d[:])
        biasg = wp.tile([128, G], F16)
        nc.sync.dma_start(biasg[:], biasg_d[:])
        biasgh_r = wp.tile([128, H], F16)
        nc.sync.dma_start(biasgh_r[:], biasgh_r_d[:])
        e0 = wp.tile([128, 128], F16)
        nc.sync.dma_start(e0[:], e0_d[:])
        eye16 = wp.tile([128, 128], F16)
        nc.sync.dma_start(eye16[:], eye_d[:])
        biass = wp.tile([1, 896], F16)
        nc.sync.dma_start(biass[:], biass_d[:])
        wgx = wp.tile([F, 1], F32)
        nc.sync.dma_start(wgx[:], wgx_d[:])
        nbgx = wp.tile([F, 1], F32)
        nc.sync.dma_start(nbgx[:], nbgx_d[:])
        ones = wp.tile([1, B], F16)
        nc.vector.memset(ones[:], 1.0)

        b_hist = biass[0:1, 0:128]
        b_feat = biass[0:1, 128:256]
        b_gh = biass[0:1, 256:768]
        b_comb = biass[0:1, 768:896]

        # ---- persistent state ----------------------------------------------
        hT = st.tile([128, H], F16)   # gamma-premultiplied h, chunked
        cT = st.tile([128, H], F16)
        nc.vector.memset(hT[:], 0.0)
        nc.vector.memset(cT[:], 0.0)

        # per-step rotating tiles, by tag
        x_in = [None] * n_steps
        m_in = [None] * n_steps
        m8_in = [None] * n_steps
        d_in = [None] * n_steps
        gam = [None] * n_steps     # gamma_h^T fp16 [128, 512]
        gamx = [None] * n_steps    # gamma_x^T fp16 [128, 128]
        alph = [None] * n_steps    # alpha^T fp16
        g_ps_t = [None] * n_steps  # gates psum (batch-major)
        xz_ps_t = [None] * n_steps
        al_ps_t = [None] * n_steps
        gam_ps_t = [None] * n_steps

        def emit_far(s):
            """input DMAs for step s + gamma_h psum matmuls for step s."""
            x_in[s] = inp.tile([F, B], F16, tag="x_in", name="x_in")
            nc.sync.dma_start(x_in[s][:], xT_d[s])
            m_in[s] = inp.tile([F, B], F16, tag="m_in", name="m_in")
            nc.sync.dma_start(m_in[s][:], mT_d[s])
            d_in[s] = inp.tile([F, B], F16, tag="d_in", name="d_in")
            nc.sync.dma_start(d_in[s][:], dT_d[s])
            m8_in[s] = inp.tile([F, B], mybir.dt.uint8, tag="m8_in", name="m8_in")
            nc.sync.dma_start(m8_in[s][:], mT8_d[s])
            if s >= 1:
                # gamma_h pre-activation, batch-major [B, 512]
                gp = ps_gam.tile([128, H], F32, tag="gamtr", name="gam_ps")
                gam_ps_t[s] = gp
                nc.tensor.matmul(gp[:], e0[:], biasgh_r[:], start=True, stop=False)
                nc.tensor.matmul(gp[:], d_in[s][:], wgh[:], start=False, stop=True)

        def emit_near_a(s):
            """input-only work for step s with no psum-bank contention:
            gamma_x chain, alpha, gamma_h activation."""
            # gamma_x chain (feature-major, per-partition scale/bias)
            y16 = wk.tile([F, B], F16, tag="y16", name="y16")
            nc.vector.tensor_scalar(y16[:], d_in[s][:], wgx[:, 0:1], None, OP.mult)
            e16 = wk.tile([F, B], F16, tag="e16", name="e16")
            nc.scalar.activation(e16[:], y16[:], AF.Exp, bias=nbgx[:, 0:1], scale=-1.0)
            gamx[s] = wk.tile([F, B], F16, tag="gamx", name="gamx")
            nc.vector.tensor_scalar(gamx[s][:], e16[:], 1.0, None, OP.min)

            # alpha matmuls are emitted mid-chain of the previous step (emit_mid)

            # gamma_h activation: min(exp(-(pre)), 1)
            if s >= 1:
                gexp = wk.tile([128, H], F16, tag="gexp", name="gexp")
                nc.scalar.activation(gexp[:], gam_ps_t[s][:], AF.Exp, scale=-1.0)
                gam[s] = wk.tile([128, H], F16, tag="gam", name="gam")
                nc.vector.tensor_scalar(gam[s][:], gexp[:], 1.0, None, OP.min)

        def emit_near_b(s):
            """psum-bank-reusing prep for step s (must be emitted after
            emit_chain(s-1) so the previous step's readers precede us)."""
            # gates psum (batch-major [B, 2048]): bias rows + m slabs
            gps = ps_g.tile([128, G], F32, tag="g_ps", name="g_ps")
            g_ps_t[s] = gps
            for c in range(4):
                nc.tensor.matmul(gps[:, c * 512:(c + 1) * 512],
                                 e0[:], biasg[:, c * 512:(c + 1) * 512],
                                 start=True, stop=False)
            for c in range(4):
                nc.tensor.matmul(gps[:, c * 512:(c + 1) * 512],
                                 m_in[s][:], wih[:, G + c * 512:G + (c + 1) * 512],
                                 start=False, stop=False)



        def emit_mid(s):
            """bias/alpha matmuls for step s, emitted mid-chain of step s-1 so
            they fill the PE stall while the previous step's DVE chain runs."""
            xz = ps_xz.tile([128, 256], F32, tag="xz_ps", name="xz_ps")
            xz_ps_t[s] = xz
            nc.tensor.matmul(xz[:, 0:128], b_hist[:], ones[:], start=True, stop=False)
            nc.tensor.matmul(xz[:, 128:256], b_feat[:], ones[:], start=False, stop=False,
                             skip_group_check=True)
            ap_ = ps_al.tile([128, B], F32, tag="al_ps", name="al_ps")
            al_ps_t[s] = ap_
            nc.tensor.matmul(ap_[:], b_comb[:], ones[:], start=True, stop=False)
            nc.tensor.matmul(ap_[:], wcomb[:, 0:F], gamx[s][:], start=False, stop=False)
            nc.tensor.matmul(ap_[:], wcomb[:, F:2 * F], m_in[s][:], start=False, stop=True)
            ta = wk.tile([F, B], F16, tag="ta", name="ta")
            nc.scalar.activation(ta[:], ap_[:], AF.Tanh, scale=0.5)
            alph[s] = wk.tile([F, B], F16, tag="alpha", name="alpha")
            nc.vector.tensor_scalar(alph[s][:], ta[:], 0.5, 0.5, OP.mult, OP.add)

        def emit_xh(s, k):
            nc.tensor.matmul(xz_ps_t[s][:, 0:128], whist[:, k * 128:(k + 1) * 128],
                             hT[:, k * 128:(k + 1) * 128], start=False, stop=(k == HC - 1))

        def emit_hg(s, k):
            for c in range(4):
                nc.tensor.matmul(g_ps_t[s][:, c * 512:(c + 1) * 512],
                                 hT[:, k * 128:(k + 1) * 128],
                                 whh[:, k * G + c * 512:k * G + (c + 1) * 512],
                                 start=False, stop=False)

        def emit_chain(s):
            gps = g_ps_t[s]
            x_h_ps = xz_ps_t[s][:, 0:128]
            z_ps = xz_ps_t[s][:, 128:256]

            if s == 0:
                for k in range(HC):
                    emit_xh(0, k)
                for k in range(HC):
                    emit_hg(0, k)

            # x_h evacs: fp16 via ACT (for compute), fp32 via ACT (for output)
            x_h16 = wk.tile([F, B], F16, tag="x_h16", name="x_h16")
            nc.scalar.activation(x_h16[:], x_h_ps, AF.Copy)
            x_h32 = wk.tile([F, B], F32, tag="x_h32", name="x_h32")
            nc.scalar.activation(x_h32[:], x_h_ps, AF.Copy)
            x_c16 = wk.tile([F, B], F16, tag="x_c16", name="x_c16")
            nc.vector.tensor_copy(x_c16[:], x_h_ps)
            nc.vector.copy_predicated(x_c16[:], m8_in[s][:], x_in[s][:])

            # z_h: x_c slab (skip group check: x_h group in this bank already closed)
            nc.tensor.matmul(z_ps, wfeat[:], x_c16[:], start=False, stop=True,
                             skip_group_check=True)

            # c_h = x_h + alpha*(z - x_h); CH output in fp32
            t1 = wk.tile([F, B], F16, tag="t1", name="t1")
            nc.vector.tensor_tensor(t1[:], z_ps, x_h16[:], OP.subtract)
            t2 = wk.tile([F, B], F16, tag="t2", name="t2")
            nc.vector.tensor_mul(t2[:], alph[s][:], t1[:])
            if s + 1 < n_steps:
                emit_mid(s + 1)
            c_h16 = wk.tile([F, B], F16, tag="c_h16", name="c_h16")
            nc.vector.tensor_add(c_h16[:], x_h16[:], t2[:])

            # c_c select, gates c_c slab
            c_c16 = wk.tile([F, B], F16, tag="c_c16", name="c_c16")
            nc.vector.tensor_copy(c_c16[:], c_h16[:])
            nc.vector.copy_predicated(c_c16[:], m8_in[s][:], x_in[s][:])
            c_h32 = wk.tile([F, B], F32, tag="c_h32", name="c_h32")
            nc.vector.tensor_add(c_h32[:], x_h16[:], t2[:])
            for c in range(4):
                nc.tensor.matmul(gps[:, c * 512:(c + 1) * 512],
                                 c_c16[:], wih[:, c * 512:(c + 1) * 512],
                                 start=False, stop=True)

            # activations: sigmoid over i|f|o, tanh over g
            tfo = wk.tile([128, 3 * H], F16, tag="tfo", name="tfo")
            nc.scalar.activation(tfo[:], gps[:, 0:3 * H], AF.Tanh, scale=0.5)
            sig = wk.tile([128, 3 * H], F16, tag="sig", name="sig")
            nc.vector.tensor_scalar(sig[:, H:2 * H], tfo[:, H:2 * H], 0.5, 0.5, OP.mult, OP.add)
            nc.vector.tensor_scalar(sig[:, 0:H], tfo[:, 0:H], 0.5, 0.5, OP.mult, OP.add)
            nc.vector.tensor_scalar(sig[:, 2 * H:3 * H], tfo[:, 2 * H:3 * H], 0.5, 0.5, OP.mult, OP.add)
            tg = wk.tile([128, H], F16, tag="tg", name="tg")
            nc.scalar.activation(tg[:], gps[:, 3 * H:G], AF.Tanh)
            if s + 1 < n_steps:
                emit_near_b(s + 1)

            # c_new = sig_f*c + sig_i*tanh_g
            fc = wk.tile([128, H], F16, tag="fc", name="fc")
            nc.vector.tensor_mul(fc[:], sig[:, H:2 * H], cT[:])
            ig = wk.tile([128, H], F16, tag="ig", name="ig")
            nc.vector.tensor_mul(ig[:], sig[:, 0:H], tg[:])
            nc.vector.tensor_add(cT[:], fc[:], ig[:])

            # h_new: batch-major s = sig_o * tanh(c), then PE-transpose back to
            # feature-major with the gamma(t+1) multiply fused into the evacuation
            thc = wk.tile([128, H], F16, tag="thc", name="thc")
            nc.scalar.activation(thc[:], cT[:], AF.Tanh)
            if s < n_steps - 1:
                s_pre = wk.tile([128, H], F16, tag="s_pre", name="s_pre")
                nc.vector.tensor_mul(s_pre[:], gam[s + 1][:], sig[:, 2 * H:3 * H])
                s_bm = wk.tile([128, H], F16, tag="s_bm", name="s_bm")
                trA = ps_gam.tile([128, 256], F16, tag="gamtr", name="trA")
                trB = ps_tr2.tile([128, 256], F16, tag="tr2", name="trB")
                trs = [(trA, 0), (trB, 0), (trA, 1), (trB, 1)]
                for k in range(HC):
                    nc.vector.tensor_mul(s_bm[:, k * 128:(k + 1) * 128],
                                         thc[:, k * 128:(k + 1) * 128],
                                         s_pre[:, k * 128:(k + 1) * 128])
                    tr, half = trs[k]
                    nc.tensor.matmul(tr[:, half * 128:(half + 1) * 128],
                                     s_bm[:, k * 128:(k + 1) * 128], eye16[:],
                                     is_transpose=True, start=(half == 0), stop=True,
                                     skip_group_check=(half == 1))
                for k in range(HC):
                    tr, half = trs[k]
                    nc.vector.tensor_copy(hT[:, k * 128:(k + 1) * 128],
                                          tr[:, half * 128:(half + 1) * 128])
                    emit_xh(s + 1, k)
                for k in range(HC):
                    emit_hg(s + 1, k)
            else:
                h32 = wk.tile([128, H], F32, tag="h32", name="h32")
                nc.vector.tensor_mul(h32[:], thc[:], sig[:, 2 * H:3 * H])
                nc.sync.dma_start(HT_d[:], h32[:])

            # fp32 output evacuations + DMA out
            z32 = wk.tile([F, B], F32, tag="z32", name="z32")
            nc.scalar.activation(z32[:], z_ps, AF.Copy)
            nc.sync.dma_start(XH_d[s], x_h32[:])
            nc.sync.dma_start(ZH_d[s], z32[:])
            nc.sync.dma_start(CH_d[s], c_h32[:])

        # ---- program ---------------------------------------------------------
        emit_far(0)
        if n_steps > 1:
            emit_far(1)
        emit_near_a(0)
        emit_near_b(0)
        emit_mid(0)
        for t in range(n_steps):
            if t + 1 < n_steps:
                emit_near_a(t + 1)
            emit_chain(t)
            if t + 2 < n_steps:
                emit_far(t + 2)

    nc.compile()
    return nc


# ---- host-side prep ---------------------------------------------------------

def _prep_shared(W_gh, b_gh, W_gx, b_gx, W_hist, b_hist, W_feat, b_feat,
                 W_comb, b_comb, W_ih, W_hh, b_ih, b_hh):
    f16 = np.float16
    eye = np.eye(F, dtype=np.float32)
    perm = np.r_[0:512, 512:1024, 1536:2048, 1024:1536]  # [i,f,o,g]

    def slabify(WT, nslab):  # [nslab*128, N] -> [128, nslab*N]
        n = WT.shape[1]
        return np.ascontiguousarray(
            WT.reshape(nslab, 128, n).transpose(1, 0, 2).reshape(128, nslab * n))

    whh = slabify(np.ascontiguousarray(W_hh[perm].T), HC).astype(f16)
    wih = slabify(np.ascontiguousarray(W_ih[perm].T), 2).astype(f16)
    whist = slabify(np.ascontiguousarray(W_hist.T), HC).astype(f16)
    wgh = np.ascontiguousarray(W_gh.T).astype(f16)
    wfeat = np.ascontiguousarray((W_feat * (1.0 - eye)).T).astype(f16)
    wcomb = slabify(np.ascontiguousarray(W_comb.T), 2).astype(f16)
    biasg = np.zeros((128, G), np.float32)
    biasg[0] = (b_ih + b_hh)[perm]
    biasg = biasg.astype(f16)
    biasgh_r = np.zeros((128, H), np.float32)
    biasgh_r[0] = b_gh
    biasgh_r = biasgh_r.astype(f16)
    e0 = np.zeros((128, 128), np.float32)
    e0[0] = 1.0
    e0 = e0.astype(f16)
    eye16 = np.eye(128, dtype=np.float32).astype(f16)
    biass = np.concatenate([b_hist, b_feat, b_gh, b_comb]).reshape(1, 896).astype(f16)
    wgx = np.ascontiguousarray(np.diag(W_gx)).reshape(F, 1).astype(np.float32)
    nbgx = (-b_gx).reshape(F, 1).astype(np.float32)
    return dict(whh=whh, wih=wih, whist=whist, wgh=wgh, wfeat=wfeat,
                wcomb=wcomb, biasg=biasg, biasgh_r=biasgh_r, e0=e0,
                eye16=eye16, biass=biass, wgx=wgx, nbgx=nbgx)


_NC_CACHE = {}


def kernel(X, missing_mask, deltas, W_gh, b_gh, W_gx, b_gx, W_hist, b_hist,
           W_feat, b_feat, W_comb, b_comb, W_ih, W_hh, b_ih, b_hh):
    X = np.asarray(X, np.float32)
    missing_mask = np.asarray(missing_mask, np.float32)
    deltas = np.asarray(deltas, np.float32)

    n_steps = X.shape[1]
    if n_steps not in _NC_CACHE:
        _NC_CACHE[n_steps] = build_nc(n_steps)
    nc = _NC_CACHE[n_steps]

    shared = _prep_shared(np.asarray(W_gh, np.float32), np.asarray(b_gh, np.float32),
                          np.asarray(W_gx, np.float32), np.asarray(b_gx, np.float32),
                          np.asarray(W_hist, np.float32), np.asarray(b_hist, np.float32),
                          np.asarray(W_feat, np.float32), np.asarray(b_feat, np.float32),
                          np.asarray(W_comb, np.float32), np.asarray(b_comb, np.float32),
                          np.asarray(W_ih, np.float32), np.asarray(W_hh, np.float32),
                          np.asarray(b_ih, np.float32), np.asarray(b_hh, np.float32))

    n_cores = 8
    bt = X.shape[0] // n_cores
    in_maps = []
    for c in range(n_cores):
        sl = slice(c * bt, (c + 1) * bt)
        in_maps.append(dict(
            xT=np.ascontiguousarray(X[sl].transpose(1, 2, 0)).astype(np.float16),
            mT=np.ascontiguousarray(missing_mask[sl].transpose(1, 2, 0)).astype(np.float16),
            mT8=np.ascontiguousarray(missing_mask[sl].transpose(1, 2, 0)).astype(np.uint8),
            dT=np.ascontiguousarray(deltas[sl].transpose(1, 2, 0)).astype(np.float16),
            **shared,
        ))

    res = run_bass_kernel_spmd(nc, in_maps, core_ids=list(range(n_cores)))

    Bfull = X.shape[0]
    XH = np.empty((Bfull, n_steps, F), np.float32)
    ZH = np.empty_like(XH)
    CH = np.empty_like(XH)
    h_T = np.empty((Bfull, H), np.float32)
    for c in range(n_cores):
        r = res.results[c]
        sl = slice(c * bt, (c + 1) * bt)
        XH[sl] = r["XH"].transpose(2, 0, 1)
        ZH[sl] = r["ZH"].transpose(2, 0, 1)
        CH[sl] = r["CH"].transpose(2, 0, 1)
        h_T[sl] = r["HT"]

    imputed = missing_mask * X + (1.0 - missing_mask) * CH
    return imputed, CH, h_T, XH, CH, ZH


# revision 15
# speedup vs baseline: 1.2152x; 1.1792x over previous
"""Trainium2 Bass kernel for nn_BackboneRITS (RITS recurrent imputation cell).

Strategy:
  - Data-parallel over batch: B=1024 -> 128 per core x 8 cores; weights replicated.
  - Everything on-device is *feature-major* ([feature, batch] per timestep), so
    LSTM/RITS matmuls need no transposes anywhere: out_chunk[Mfeat,B] =
    W_slab[K,M].T @ act[K,B], with fp16 operands (1 cyc/row on PE) and fp32 PSUM
    accumulation.  Biases are injected with K=1 rank-1 matmuls (bias x ones).
  - gamma_h / gamma_x / alpha depend only on inputs (d, m) -> computed off the
    critical recurrence chain, pipelined one step ahead.
  - Gate rows are pre-permuted [i,f,o,g] so one batched Sigmoid covers i,f,o.
  - imputed == c_c == m*x + (1-m)*c_h, so it is reassembled on the host from CH
    (exact elementwise, no extra device work).
"""

import os
import sys

sys.path.insert(0, "/opt/trn_rl_repo")
os.environ.setdefault("MYCRO_LOCAL_CACHE", "1")

import numpy as np

import concourse.bacc as bacc
import concourse.tile as tile
from concourse import mybir
from concourse.bass_utils import run_bass_kernel_spmd

B = 128          # batch per core
T = 128          # timesteps
F = 128          # features
H = 512          # hidden
G = 2048         # 4*H gate width
HC = H // 128    # hidden chunks
GC = G // 128    # gate chunks

F16 = mybir.dt.float16
F32 = mybir.dt.float32
AF = mybir.ActivationFunctionType
OP = mybir.AluOpType


def build_nc(n_steps=T):
    nc = bacc.Bacc("TRN2", target_bir_lowering=False, debug=False,
                   enable_asserts=True, num_devices=8)

    # ---- DRAM tensors -------------------------------------------------------
    def din(name, shape, dt=F16):
        return nc.dram_tensor(name, shape, dt, kind="ExternalInput").ap()

    def dout(name, shape, dt=F32):
        return nc.dram_tensor(name, shape, dt, kind="ExternalOutput").ap()

    xT_d = din("xT", [n_steps, F, B])
    mT_d = din("mT", [n_steps, F, B])
    dT_d = din("dT", [n_steps, F, B])
    mT8_d = din("mT8", [n_steps, F, B], mybir.dt.uint8)
    whh_d = din("whh", [128, HC * G])      # W_hh.T row-slabs side by side
    wih_d = din("wih", [128, 2 * G])       # W_ih.T slabs (c_c | m)
    whist_d = din("whist", [128, HC * F])  # W_hist.T row-slabs
    wgh_d = din("wgh", [128, H])           # W_gh.T
    wfeat_d = din("wfeat", [128, F])       # (W_feat*(1-I)).T
    wcomb_d = din("wcomb", [128, 2 * F])   # W_comb.T slabs (gamma_x | m)
    biasg_d = din("biasg", [128, G])       # row0 = (b_ih+b_hh)[perm], rest 0
    biasgh_r_d = din("biasgh_r", [128, H]) # row0 = b_gh, rest 0
    e0_d = din("e0", [128, 128])           # row0 = ones, rest 0
    eye_d = din("eye16", [128, 128])       # fp16 identity for PE transpose
    biass_d = din("biass", [1, 896])       # b_hist | b_feat | b_gh(512) | b_comb
    wgx_d = din("wgx", [F, 1], F32)        # diag(W_gx) per-partition column
    nbgx_d = din("nbgx", [F, 1], F32)      # -b_gx per-partition column

    XH_d = dout("XH", [n_steps, F, B])
    ZH_d = dout("ZH", [n_steps, F, B])
    CH_d = dout("CH", [n_steps, F, B])
    HT_d = dout("HT", [128, H])            # final h, chunked feature-major

    from contextlib import ExitStack
    with tile.TileContext(nc) as tc, ExitStack() as ctx:
        wp = ctx.enter_context(tc.tile_pool(name="weights", bufs=1))
        inp = ctx.enter_context(tc.tile_pool(name="inputs", bufs=4))
        st = ctx.enter_context(tc.tile_pool(name="state", bufs=1))
        wk = ctx.enter_context(tc.tile_pool(name="work", bufs=2))
        ps_g = ctx.enter_context(tc.tile_pool(name="ps_gates", bufs=1, space="PSUM"))
        ps_gam = ctx.enter_context(tc.tile_pool(name="ps_gamtr", bufs=1, space="PSUM"))
        ps_xz = ctx.enter_context(tc.tile_pool(name="ps_xz", bufs=1, space="PSUM"))
        ps_tr2 = ctx.enter_context(tc.tile_pool(name="ps_tr2", bufs=1, space="PSUM"))
        ps_al = ctx.enter_context(tc.tile_pool(name="ps_alpha", bufs=1, space="PSUM"))

        # ---- resident weights ----------------------------------------------
        whh = wp.tile([128, HC * G], F16)
        nc.sync.dma_start(whh[:], whh_d[:])
        wih = wp.tile([128, 2 * G], F16)
        nc.sync.dma_start(wih[:], wih_d[:])
        whist = wp.tile([128, HC * F], F16)
        nc.sync.dma_start(whist[:], whist_d[:])
        wgh = wp.tile([128, H], F16)
        nc.sync.dma_start(wgh[:], wgh_d[:])
        wfeat = wp.tile([128, F], F16)
        nc.sync.dma_start(wfeat[:], wfeat_d[:])
        wcomb = wp.tile([128, 2 * F], F16)
        nc.sync.dma_start(wcomb[:], wcomb_d[:])
        biasg = wp.tile([128, G], F16)
        nc.sync.dma_start(biasg[:], biasg_d[:])
        biasgh_r = wp.tile([128, H], F16)
        nc.sync.dma_start(biasgh_r[:], biasgh_r_d[:])
        e0 = wp.tile([128, 128], F16)
        nc.sync.dma_start(e0[:], e0_d[:])
        eye16 = wp.tile([128, 128], F16)
        nc.sync.dma_start(eye16[:], eye_d[:])
        biass = wp.tile([1, 896], F16)
        nc.sync.dma_start(biass[:], biass_d[:])
        wgx = wp.tile([F, 1], F32)
        nc.sync.dma_start(wgx[:], wgx_d[:])
        nbgx = wp.tile([F, 1], F32)
        nc.sync.dma_start(nbgx[:], nbgx_d[:])
        ones = wp.tile([1, B], F16)
        nc.vector.memset(ones[:], 1.0)

        b_hist = biass[0:1, 0:128]
        b_feat = biass[0:1, 128:256]
        b_gh = biass[0:1, 256:768]
        b_comb = biass[0:1, 768:896]

        # ---- persistent state ----------------------------------------------
        hT = st.tile([128, H], F16)   # gamma-premultiplied h, chunked
        cT = st.tile([128, H], F16)
        nc.vector.memset(hT[:], 0.0)
        nc.vector.memset(cT[:], 0.0)

        # per-step rotating tiles, by tag
        x_in = [None] * n_steps
        m_in = [None] * n_steps
        m8_in = [None] * n_steps
        d_in = [None] * n_steps
        gam = [None] * n_steps     # gamma_h^T fp16 [128, 512]
        gamx = [None] * n_steps    # gamma_x^T fp16 [128, 128]
        alph = [None] * n_steps    # alpha^T fp16
        g_ps_t = [None] * n_steps  # gates psum (batch-major)
        xz_ps_t = [None] * n_steps
        al_ps_t = [None] * n_steps
        gam_ps_t = [None] * n_steps

        def emit_far(s):
            """input DMAs for step s + gamma_h psum matmuls for step s."""
            x_in[s] = inp.tile([F, B], F16, tag="x_in", name="x_in")
            nc.sync.dma_start(x_in[s][:], xT_d[s])
            m_in[s] = inp.tile([F, B], F16, tag="m_in", name="m_in")
            nc.sync.dma_start(m_in[s][:], mT_d[s])
            d_in[s] = inp.tile([F, B], F16, tag="d_in", name="d_in")
            nc.sync.dma_start(d_in[s][:], dT_d[s])
            m8_in[s] = inp.tile([F, B], mybir.dt.uint8, tag="m8_in", name="m8_in")
            nc.sync.dma_start(m8_in[s][:], mT8_d[s])
            if s >= 1:
                # gamma_h pre-activation, batch-major [B, 512]
                gp = ps_gam.tile([128, H], F32, tag="gamtr", name="gam_ps")
                gam_ps_t[s] = gp
                nc.tensor.matmul(gp[:], e0[:], biasgh_r[:], start=True, stop=False)
                nc.tensor.matmul(gp[:], d_in[s][:], wgh[:], start=False, stop=True)

        def emit_near_a(s):
            """input-only work for step s with no psum-bank contention:
            gamma_x chain, alpha, gamma_h activation."""
            # gamma_x chain (feature-major, per-partition scale/bias)
            y16 = wk.tile([F, B], F16, tag="y16", name="y16")
            nc.vector.tensor_scalar(y16[:], d_in[s][:], wgx[:, 0:1], None, OP.mult)
            e16 = wk.tile([F, B], F16, tag="e16", name="e16")
            nc.scalar.activation(e16[:], y16[:], AF.Exp, bias=nbgx[:, 0:1], scale=-1.0)
            gamx[s] = wk.tile([F, B], F16, tag="gamx", name="gamx")
            nc.vector.tensor_scalar(gamx[s][:], e16[:], 1.0, None, OP.min)

            # alpha matmuls are emitted mid-chain of the previous step (emit_mid)

            # gamma_h activation: min(exp(-(pre)), 1)
            if s >= 1:
                gexp = wk.tile([128, H], F16, tag="gexp", name="gexp")
                nc.scalar.activation(gexp[:], gam_ps_t[s][:], AF.Exp, scale=-1.0)
                gam[s] = wk.tile([128, H], F16, tag="gam", name="gam")
                nc.vector.tensor_scalar(gam[s][:], gexp[:], 1.0, None, OP.min)

        def emit_near_b(s):
            """psum-bank-reusing prep for step s (must be emitted after
            emit_chain(s-1) so the previous step's readers precede us)."""
            # gates psum (batch-major [B, 2048]): bias rows + m slabs
            gps = ps_g.tile([128, G], F32, tag="g_ps", name="g_ps")
            g_ps_t[s] = gps
            for c in range(4):
                nc.tensor.matmul(gps[:, c * 512:(c + 1) * 512],
                                 e0[:], biasg[:, c * 512:(c + 1) * 512],
                                 start=True, stop=False)
            for c in range(4):
                nc.tensor.matmul(gps[:, c * 512:(c + 1) * 512],
                                 m_in[s][:], wih[:, G + c * 512:G + (c + 1) * 512],
                                 start=False, stop=False)



        def emit_mid(s):
            """bias/alpha matmuls for step s, emitted mid-chain of step s-1 so
            they fill the PE stall while the previous step's DVE chain runs."""
            xz = ps_xz.tile([128, 256], F32, tag="xz_ps", name="xz_ps")
            xz_ps_t[s] = xz
            nc.tensor.matmul(xz[:, 0:128], b_hist[:], ones[:], start=True, stop=False)
            nc.tensor.matmul(xz[:, 128:256], b_feat[:], ones[:], start=False, stop=False,
                             skip_group_check=True)
            ap_ = ps_al.tile([128, B], F32, tag="al_ps", name="al_ps")
            al_ps_t[s] = ap_
            nc.tensor.matmul(ap_[:], b_comb[:], ones[:], start=True, stop=False)
            nc.tensor.matmul(ap_[:], wcomb[:, 0:F], gamx[s][:], start=False, stop=False)
            nc.tensor.matmul(ap_[:], wcomb[:, F:2 * F], m_in[s][:], start=False, stop=True)
            ta = wk.tile([F, B], F16, tag="ta", name="ta")
            nc.scalar.activation(ta[:], ap_[:], AF.Tanh, scale=0.5)
            alph[s] = wk.tile([F, B], F16, tag="alpha", name="alpha")
            nc.vector.tensor_scalar(alph[s][:], ta[:], 0.5, 0.5, OP.mult, OP.add)

        def emit_xh(s, k):
            nc.tensor.matmul(xz_ps_t[s][:, 0:128], whist[:, k * 128:(k + 1) * 128],
                             hT[:, k * 128:(k + 1) * 128], start=False, stop=(k == HC - 1))

        def emit_hg(s, k):
            for c in range(4):
                nc.tensor.matmul(g_ps_t[s][:, c * 512:(c + 1) * 512],
                                 hT[:, k * 128:(k + 1) * 128],
                                 whh[:, k * G + c * 512:k * G + (c + 1) * 512],
                                 start=False, stop=False)

        def emit_chain(s):
            gps = g_ps_t[s]
            x_h_ps = xz_ps_t[s][:, 0:128]
            z_ps = xz_ps_t[s][:, 128:256]

            if s == 0:
                for k in range(HC):
                    emit_xh(0, k)
                emit_hg(0, 0)
                emit_hg(0, 1)

            # x_h evacs: fp16 via ACT (for compute), fp32 via ACT (for output)
            x_h16 = wk.tile([F, B], F16, tag="x_h16", name="x_h16")
            nc.scalar.activation(x_h16[:], x_h_ps, AF.Copy)
            x_h32 = wk.tile([F, B], F32, tag="x_h32", name="x_h32")
            nc.scalar.activation(x_h32[:], x_h_ps, AF.Copy)
            x_c16 = wk.tile([F, B], F16, tag="x_c16", name="x_c16")
            nc.vector.tensor_copy(x_c16[:], x_h_ps)
            nc.vector.copy_predicated(x_c16[:], m8_in[s][:], x_in[s][:])

            # z_h: x_c slab (skip group check: x_h group in this bank already closed)
            nc.tensor.matmul(z_ps, wfeat[:], x_c16[:], start=False, stop=True,
                             skip_group_check=True)
            emit_hg(s, 2)
            emit_hg(s, 3)

            # c_h = x_h + alpha*(z - x_h); CH output in fp32
            t1 = wk.tile([F, B], F16, tag="t1", name="t1")
            nc.vector.tensor_tensor(t1[:], z_ps, x_h16[:], OP.subtract)
            t2 = wk.tile([F, B], F16, tag="t2", name="t2")
            nc.vector.tensor_mul(t2[:], alph[s][:], t1[:])
            if s + 1 < n_steps:
                emit_mid(s + 1)
            c_h16 = wk.tile([F, B], F16, tag="c_h16", name="c_h16")
            nc.vector.tensor_add(c_h16[:], x_h16[:], t2[:])

            # c_c select, gates c_c slab
            c_c16 = wk.tile([F, B], F16, tag="c_c16", name="c_c16")
            nc.vector.tensor_copy(c_c16[:], c_h16[:])
            nc.vector.copy_predicated(c_c16[:], m8_in[s][:], x_in[s][:])
            c_h32 = wk.tile([F, B], F32, tag="c_h32", name="c_h32")
            nc.vector.tensor_add(c_h32[:], x_h16[:], t2[:])
            for c in range(4):
                nc.tensor.matmul(gps[:, c * 512:(c + 1) * 512],
                                 c_c16[:], wih[:, c * 512:(c + 1) * 512],
                                 start=False, stop=True)

            # activations: sigmoid over i|f|o, tanh over g
            tfo = wk.tile([128, 3 * H], F16, tag="tfo", name="tfo")
            nc.scalar.activation(tfo[:], gps[:, 0:3 * H], AF.Tanh, scale=0.5)
            sig = wk.tile([128, 3 * H], F16, tag="sig", name="sig")
            nc.vector.tensor_scalar(sig[:, H:2 * H], tfo[:, H:2 * H], 0.5, 0.5, OP.mult, OP.add)
            tg = wk.tile([128, H], F16, tag="tg", name="tg")
            nc.scalar.activation(tg[:], gps[:, 3 * H:G], AF.Tanh)
            if s + 1 < n_steps:
                emit_near_b(s + 1)

            # c_new = sig_f*c + sig_i*tanh_g
            fc = wk.tile([128, H], F16, tag="fc", name="fc")
            nc.vector.tensor_mul(fc[:], sig[:, H:2 * H], cT[:])
            nc.vector.tensor_scalar(sig[:, 0:H], tfo[:, 0:H], 0.5, 0.5, OP.mult, OP.add)
            ig = wk.tile([128, H], F16, tag="ig", name="ig")
            nc.vector.tensor_mul(ig[:], sig[:, 0:H], tg[:])
            nc.vector.tensor_scalar(sig[:, 2 * H:3 * H], tfo[:, 2 * H:3 * H], 0.5, 0.5, OP.mult, OP.add)
            nc.vector.tensor_add(cT[:], fc[:], ig[:])

            # h_new: batch-major s = sig_o * tanh(c), then PE-transpose back to
            # feature-major with the gamma(t+1) multiply fused into the evacuation
            thc = wk.tile([128, H], F16, tag="thc", name="thc")
            nc.scalar.activation(thc[:], cT[:], AF.Tanh)
            if s < n_steps - 1:
                s_pre = wk.tile([128, H], F16, tag="s_pre", name="s_pre")
                nc.vector.tensor_mul(s_pre[:], gam[s + 1][:], sig[:, 2 * H:3 * H])
                s_bm = wk.tile([128, H], F16, tag="s_bm", name="s_bm")
                trA = ps_gam.tile([128, 256], F16, tag="gamtr", name="trA")
                trB = ps_tr2.tile([128, 256], F16, tag="tr2", name="trB")
                trs = [(trA, 0), (trB, 0), (trA, 1), (trB, 1)]
                for k in range(HC):
                    nc.vector.tensor_mul(s_bm[:, k * 128:(k + 1) * 128],
                                         thc[:, k * 128:(k + 1) * 128],
                                         s_pre[:, k * 128:(k + 1) * 128])
                    tr, half = trs[k]
                    nc.tensor.matmul(tr[:, half * 128:(half + 1) * 128],
                                     s_bm[:, k * 128:(k + 1) * 128], eye16[:],
                                     is_transpose=True, start=(half == 0), stop=True,
                                     skip_group_check=(half == 1))
                for k in range(HC):
                    tr, half = trs[k]
                    nc.vector.tensor_copy(hT[:, k * 128:(k + 1) * 128],
                                          tr[:, half * 128:(half + 1) * 128])
                    emit_xh(s + 1, k)
                emit_hg(s + 1, 0)
                emit_hg(s + 1, 1)
            else:
                h32 = wk.tile([128, H], F32, tag="h32", name="h32")
                nc.vector.tensor_mul(h32[:], thc[:], sig[:, 2 * H:3 * H])
                nc.sync.dma_start(HT_d[:], h32[:])

            # fp32 output evacuations + DMA out
            z32 = wk.tile([F, B], F32, tag="z32", name="z32")
            nc.scalar.activation(z32[:], z_ps, AF.Copy)
            nc.sync.dma_start(XH_d[s], x_h32[:])
            nc.sync.dma_start(ZH_d[s], z32[:])
            nc.sync.dma_start(CH_d[s], c_h32[:])

        # ---- program ---------------------------------------------------------
        emit_far(0)
        if n_steps > 1:
            emit_far(1)
        emit_near_a(0)
        emit_near_b(0)
        emit_mid(0)
        for t in range(n_steps):
            if t + 1 < n_steps:
                emit_near_a(t + 1)
            emit_chain(t)
            if t + 2 < n_steps:
                emit_far(t + 2)

    nc.compile()
    return nc


# ---- host-side prep ---------------------------------------------------------

def _prep_shared(W_gh, b_gh, W_gx, b_gx, W_hist, b_hist, W_feat, b_feat,
                 W_comb, b_comb, W_ih, W_hh, b_ih, b_hh):
    f16 = np.float16
    eye = np.eye(F, dtype=np.float32)
    perm = np.r_[0:512, 512:1024, 1536:2048, 1024:1536]  # [i,f,o,g]

    def slabify(WT, nslab):  # [nslab*128, N] -> [128, nslab*N]
        n = WT.shape[1]
        return np.ascontiguousarray(
            WT.reshape(nslab, 128, n).transpose(1, 0, 2).reshape(128, nslab * n))

    whh = slabify(np.ascontiguousarray(W_hh[perm].T), HC).astype(f16)
    wih = slabify(np.ascontiguousarray(W_ih[perm].T), 2).astype(f16)
    whist = slabify(np.ascontiguousarray(W_hist.T), HC).astype(f16)
    wgh = np.ascontiguousarray(W_gh.T).astype(f16)
    wfeat = np.ascontiguousarray((W_feat * (1.0 - eye)).T).astype(f16)
    wcomb = slabify(np.ascontiguousarray(W_comb.T), 2).astype(f16)
    biasg = np.zeros((128, G), np.float32)
    biasg[0] = (b_ih + b_hh)[perm]
    biasg = biasg.astype(f16)
    biasgh_r = np.zeros((128, H), np.float32)
    biasgh_r[0] = b_gh
    biasgh_r = biasgh_r.astype(f16)
    e0 = np.zeros((128, 128), np.float32)
    e0[0] = 1.0
    e0 = e0.astype(f16)
    eye16 = np.eye(128, dtype=np.float32).astype(f16)
    biass = np.concatenate([b_hist, b_feat, b_gh, b_comb]).reshape(1, 896).astype(f16)
    wgx = np.ascontiguousarray(np.diag(W_gx)).reshape(F, 1).astype(np.float32)
    nbgx = (-b_gx).reshape(F, 1).astype(np.float32)
    return dict(whh=whh, wih=wih, whist=whist, wgh=wgh, wfeat=wfeat,
                wcomb=wcomb, biasg=biasg, biasgh_r=biasgh_r, e0=e0,
                eye16=eye16, biass=biass, wgx=wgx, nbgx=nbgx)


_NC_CACHE = {}


def kernel(X, missing_mask, deltas, W_gh, b_gh, W_gx, b_gx, W_hist, b_hist,
           W_feat, b_feat, W_comb, b_comb, W_ih, W_hh, b_ih, b_hh):
    X = np.asarray(X, np.float32)
    missing_mask = np.asarray(missing_mask, np.float32)
    deltas = np.asarray(deltas, np.float32)

    n_steps = X.shape[1]
    if n_steps not in _NC_CACHE:
        _NC_CACHE[n_steps] = build_nc(n_steps)
    nc = _NC_CACHE[n_steps]

    shared = _prep_shared(np.asarray(W_gh, np.float32), np.asarray(b_gh, np.float32),
                          np.asarray(W_gx, np.float32), np.asarray(b_gx, np.float32),
                          np.asarray(W_hist, np.float32), np.asarray(b_hist, np.float32),
                          np.asarray(W_feat, np.float32), np.asarray(b_feat, np.float32),
                          np.asarray(W_comb, np.float32), np.asarray(b_comb, np.float32),
                          np.asarray(W_ih, np.float32), np.asarray(W_hh, np.float32),
                          np.asarray(b_ih, np.float32), np.asarray(b_hh, np.float32))

    n_cores = 8
    bt = X.shape[0] // n_cores
    in_maps = []
    for c in range(n_cores):
        sl = slice(c * bt, (c + 1) * bt)
        in_maps.append(dict(
            xT=np.ascontiguousarray(X[sl].transpose(1, 2, 0)).astype(np.float16),
            mT=np.ascontiguousarray(missing_mask[sl].transpose(1, 2, 0)).astype(np.float16),
            mT8=np.ascontiguousarray(missing_mask[sl].transpose(1, 2, 0)).astype(np.uint8),
            dT=np.ascontiguousarray(deltas[sl].transpose(1, 2, 0)).astype(np.float16),
            **shared,
        ))

    res = run_bass_kernel_spmd(nc, in_maps, core_ids=list(range(n_cores)))

    Bfull = X.shape[0]
    XH = np.empty((Bfull, n_steps, F), np.float32)
    ZH = np.empty_like(XH)
    CH = np.empty_like(XH)
    h_T = np.empty((Bfull, H), np.float32)
    for c in range(n_cores):
        r = res.results[c]
        sl = slice(c * bt, (c + 1) * bt)
        XH[sl] = r["XH"].transpose(2, 0, 1)
        ZH[sl] = r["ZH"].transpose(2, 0, 1)
        CH[sl] = r["CH"].transpose(2, 0, 1)
        h_T[sl] = r["HT"]

    imputed = missing_mask * X + (1.0 - missing_mask) * CH
    return imputed, CH, h_T, XH, CH, ZH
